# revision 48
# baseline (speedup 1.0000x reference)
"""BigBird sparse attention kernel for 8 Trainium2 NeuronCores (v4).

Sharding: token-parallel. B=2 batches x 4 chunks of 1024 local tokens each
-> 8 cores. Each core receives a transposed bf16 x-slice [D=1024, 1284] whose
columns are [g0, g1, 10 window blocks of 128 tokens, 0.5*g0, 0.5*g1]
(blocks 8j-1 .. 8j+8, zero-padded outside [0, 32)). The core computes:
  - q/k projections in transposed layout [f, tok] (bf16 matmuls, unscaled;
    the 1/sqrt(dk) softmax scale is folded into the ACT exp/tanh scale)
  - v projection in [tok, f] layout; the 0.5-scaled global columns produce
    vs=(v0+v1)/2 and vd=(v1-v0)/2 rows used by the tanh global path
  - 3-block sliding-window attention with scores kept transposed [kt, q];
    edge masking is folded into the exp ACT as a per-partition bias (-30
    for the pad block on edge cores, 0 elsewhere)
  - P.V with a ones-column denominator; 1/den via reciprocal_approx_fast on
    the single den partition, then broadcast to 64 partitions with one
    stride-0 SBUF->SBUF DMA (replaces 4 stream_shuffles/head)
  - attention of local tokens to the 2 global tokens via the 2-way-softmax
    identity out_xg = vs + tanh((s1-s0)/2)*vd  (no separate normalization)
  - flash-style partial stats (sum-exp, weighted V) of the 2 global query
    tokens against the core's local keys -> combined on host; the global
    scores are computed pair-batched with a block-diagonal qg stationary
  - output projection + bias for its 1024 local tokens
Host assembles the 8 slices, and computes the 2 global output rows per
batch exactly in numpy from the shipped partials. Weights are shipped in
a [blk, partition, sub, col] layout so every SBUF tile loads with ONE fat
DMA (2KB+ contiguous per partition row). x-slice DMAs issue on the
Activation HWDGE ring so they run concurrently with the weight DMAs on
the SP ring.
"""

import numpy as np
import ml_dtypes

import concourse.bass as bass
import concourse.mybir as mybir
import concourse.tile as tile
from concourse import bacc
from concourse.bass_utils import run_bass_kernel_spmd

F32 = mybir.dt.float32
BF16 = mybir.dt.bfloat16
AF = mybir.ActivationFunctionType
ALU = mybir.AluOpType
BF = ml_dtypes.bfloat16

D_MODEL = 1024
H = 16
DK = 64
BS = 128
B = 2
T = 4098
NB = 32            # global 128-blocks of local tokens
NW = 10            # window blocks per core (8 local + 2 halo)
TOKS = 2 + NW * BS # x-slice main columns
XCOLS = TOKS + 2   # + 2 half-scaled global columns for the v projection
LQ0 = 2 + BS       # first local-q column
SCALE = 1.0 / np.sqrt(DK)

# token chunks for the projection moving dim; q needs only the 2 global
# columns plus its 8 own blocks (cols 130..1153); k needs all 10 blocks
KCH = [(0, 512), (512, 512), (1024, 258)]
QCH = [(0, 2), (130, 512), (642, 512)]


def C(t):
    return 2 + BS * t


# P.V accumulation schedule per psum bank: (t, qstart, nblocks, start, stop)
# bank 0 covers q window-positions 1..4, bank 1 covers 5..8.
PV_SCHED = [
    [(2, 1, 3, True, False), (3, 2, 2, False, False), (3, 4, 1, False, False),
     (0, 1, 1, False, False), (1, 1, 2, False, False), (4, 3, 2, False, False),
     (5, 4, 1, False, True)],
    [(6, 5, 3, True, False), (7, 6, 2, False, False), (7, 8, 1, False, False),
     (4, 5, 1, False, False), (5, 5, 2, False, False), (8, 7, 2, False, False),
     (9, 8, 1, False, True)],
]


def ptcol(t, qpos):
    # column of (window-block t, q window-position qpos) in the pt tensor
    return 384 * t + 128 * (qpos - (t - 1))


def flat2(ap):
    """Collapse a [P, a, b] AP with contiguous (a, b) into [P, a*b]."""
    dims = ap.ap
    assert len(dims) == 3 and dims[2][0] == 1 and dims[1][0] == dims[2][1]
    return bass.AP(ap.tensor, ap.offset,
                   [list(dims[0]), [1, dims[1][1] * dims[2][1]]])


def build_kernel(nc):
    xt = nc.dram_tensor("xt", [D_MODEL, XCOLS], BF16, kind="ExternalInput").ap()
    wq = nc.dram_tensor("wq", [8, 128, 8, 128], BF16, kind="ExternalInput").ap()
    wk = nc.dram_tensor("wk", [8, 128, 8, 128], BF16, kind="ExternalInput").ap()
    wv = nc.dram_tensor("wv", [2, 128, 8, 512], BF16, kind="ExternalInput").ap()
    wo = nc.dram_tensor("wo", [8, 128, 8, 128], BF16, kind="ExternalInput").ap()
    bo = nc.dram_tensor("bo", [128, 8], F32, kind="ExternalInput").ap()
    maskl = nc.dram_tensor("maskl", [128, 1], F32, kind="ExternalInput").ap()
    maskr = nc.dram_tensor("maskr", [128, 1], F32, kind="ExternalInput").ap()
    outt = nc.dram_tensor("outt", [D_MODEL, 1024], F32, kind="ExternalOutput").ap()
    gstats = nc.dram_tensor("gstats", [2, 1536], F32, kind="ExternalOutput").ap()

    with tile.TileContext(nc) as tc:
        with (
            tc.tile_pool(name="pc", bufs=1) as pc,
            tc.tile_pool(name="px", bufs=1) as px,
            tc.tile_pool(name="pqk", bufs=2) as pqk,
            tc.tile_pool(name="pv", bufs=1) as pvp,
            tc.tile_pool(name="pwv", bufs=2) as pwv,
            tc.tile_pool(name="pw", bufs=4) as pw,
            tc.tile_pool(name="pat", bufs=1) as pat,
            tc.tile_pool(name="ppt", bufs=3) as ppt,
            tc.tile_pool(name="psm", bufs=3) as psm,
            tc.tile_pool(name="ppair", bufs=2) as ppair,
            tc.tile_pool(name="pout", bufs=2) as pout,
            tc.tile_pool(name="pps", bufs=1, space="PSUM") as pps,
        ):
            # ---- x slice, transposed, resident: one fat DMA per d-block,
            # issued on the Activation HWDGE ring (parallel to weights) ----
            xts = []
            for d in range(8):
                xd = px.tile([128, XCOLS], BF16, tag=f"xt{d}")
                nc.scalar.dma_start(xd[:], xt[128 * d:128 * (d + 1), :])
                xts.append(xd)

            # ---- constants (mask DMAs ride the ACT ring behind x so the
            # SP ring starts with the first weight tiles; the bo DMA is
            # emitted late, right before the output projection) ----
            ml_sb = pc.tile([128, 1], F32, tag="ml")
            mr_sb = pc.tile([128, 1], F32, tag="mr")
            nc.scalar.dma_start(ml_sb[:], maskl)
            nc.scalar.dma_start(mr_sb[:], maskr)
            gst = pc.tile([2, 1536], F32, tag="gst")
            nc.vector.memset(gst[:], 0.0)
            ident1 = pc.tile([1, 1], BF16, tag="ident1")
            nc.vector.memset(ident1[:], 1.0)

            at_sb = [pat.tile([128, 1024], BF16, tag=f"at{f}", name=f"at{f}")
                     for f in range(8)]

            def emit_qk_proj(pss, half, qk_tiles):
                for pname, wdram, chunks in (("q", wq, QCH), ("k", wk, KCH)):
                    for i2 in range(2):
                        i = 2 * half + i2
                        osb = pqk.tile([128, TOKS], BF16, tag=f"qk{pname}{i}",
                                       name=f"qk{pname}{i}")
                        qk_tiles[(pname, i)] = osb
                        wtb = pw.tile([128, 1024], BF16, tag="w", bufs=4)
                        nc.sync.dma_start(wtb[:],
                                          flat2(wdram[4 * pss + 2 * half + i2]))
                        for c, (c0, cn) in enumerate(chunks):
                            psj = pps.tile([128, cn], F32, tag="psj",
                                           name=f"pj{c}", bufs=2)
                            for d in range(8):
                                nc.tensor.matmul(
                                    psj[:, :cn], wtb[:, 128 * d:128 * d + 128],
                                    xts[d][:, c0:c0 + cn],
                                    start=(d == 0), stop=(d == 7))
                            nc.vector.tensor_copy(osb[:, c0:c0 + cn],
                                                  psj[:, :cn])

            def emit_v_proj(pss):
                wvb = pwv.tile([128, 4096], BF16, tag="wvb", name="wvb")
                nc.sync.dma_start(wvb[:], flat2(wv[pss]))
                v96 = []
                for tb in range(NW):
                    pv_ps = pps.tile([128, 512], F32, tag="psj", name="pv_ps",
                                     bufs=2)
                    for d in range(8):
                        nc.tensor.matmul(pv_ps[:], xts[d][:, C(tb):C(tb) + 128],
                                         wvb[:, 512 * d:512 * d + 512],
                                         start=(d == 0), stop=(d == 7))
                    vt = pvp.tile([128, 8 * 96], BF16, tag=f"v96_{tb}",
                                  name=f"v96_{tb}")
                    pstep = vt.ap[0][0]
                    dst = bass.AP(vt.tensor, vt[:].offset,
                                  [[pstep, 128], [96, 8], [1, 64]])
                    src = bass.AP(pv_ps.tensor, pv_ps[:].offset,
                                  [[pv_ps.ap[0][0], 128], [64, 8], [1, 64]])
                    nc.vector.tensor_copy(dst, src)
                    onesap = bass.AP(vt.tensor, vt[:].offset + 64,
                                     [[pstep, 128], [96, 8], [1, 1]])
                    nc.vector.memset(onesap, 1.0)
                    zap = bass.AP(vt.tensor, vt[:].offset + 65,
                                  [[pstep, 128], [96, 8], [1, 31]])
                    nc.vector.memset(zap, 0.0)
                    v96.append(vt)
                # vs/vd rows from the 0.5-scaled global columns:
                # vs = (v0+v1)/2, vd = (v1-v0)/2 (pvg inputs pre-halved).
                pvgs = []
                for gi in range(2):
                    pvg = pps.tile([1, 512], F32, tag="psj", name=f"pvg{gi}",
                                   bufs=2)
                    for d in range(8):
                        nc.tensor.matmul(pvg[:],
                                         xts[d][:, TOKS + gi:TOKS + gi + 1],
                                         wvb[:, 512 * d:512 * d + 512],
                                         start=(d == 0), stop=(d == 7))
                    vg = pvp.tile([1, 512], F32, tag=f"vg{gi}", name=f"vg{gi}")
                    nc.vector.tensor_copy(vg[:], pvg[:])
                    pvgs.append(vg)
                vs_row = pvp.tile([1, 512], BF16, tag="vs_row", name="vs_row")
                vd_row = pvp.tile([1, 512], BF16, tag="vd_row", name="vd_row")
                nc.vector.tensor_tensor(vs_row[:], pvgs[0][:], pvgs[1][:],
                                        ALU.add)
                nc.vector.tensor_tensor(vd_row[:], pvgs[1][:], pvgs[0][:],
                                        ALU.subtract)
                # vs as columns [64 f, 8 heads] for the epilogue add
                vs_cols = pvp.tile([64, 8], F32, tag="vs_cols", name="vs_cols")
                for hl in range(8):
                    tp = pps.tile([64, 1], BF16, tag="psj", name="tp", bufs=2)
                    nc.tensor.transpose(tp[:], vs_row[:, 64 * hl:64 * hl + 64],
                                        ident1[:])
                    nc.vector.tensor_copy(vs_cols[:, hl:hl + 1], tp[:])
                return v96, (vs_cols, vd_row)

            def emit_pair(pss, pair, qk_tiles):
                """Global-token scores (pair-batched) for the 2 global stats."""
                qt = qk_tiles[("q", pair)]
                kt = qk_tiles[("k", pair)]
                # block-diag global-q stationary [128, 4]
                qgp = ppair.tile([128, 4], BF16, tag="qgp", name="qgp")
                nc.vector.memset(qgp[:], 0.0)
                nc.vector.tensor_copy(qgp[0:64, 0:2], qt[0:64, 0:2])
                nc.vector.tensor_copy(qgp[64:128, 2:4], qt[64:128, 0:2])
                # global-q scores vs local keys: [128 kt, 4(h,g)] per t
                pg_ps = pps.tile([128, 32], F32, tag="small", name="pg_ps",
                                 bufs=1)
                for t in range(1, 9):
                    nc.tensor.matmul(pg_ps[:, 4 * (t - 1):4 * t],
                                     kt[:, C(t):C(t) + 128], qgp[:],
                                     start=True, stop=True)
                pgp = psm.tile([128, 32], BF16, tag="pgp", name="pgp", bufs=3)
                nc.scalar.activation(pgp[:], pg_ps[:], AF.Exp, scale=SCALE)
                return pgp

            def emit_head(h, qk_tiles, v96, vrows, pgp):
                hl = h % 8
                pair = hl // 2
                parity = hl % 2
                r0 = 64 * parity
                qt = qk_tiles[("q", pair)]
                kt = qk_tiles[("k", pair)]
                qh = qt[r0:r0 + 64, :]
                kh = kt[r0:r0 + 64, :]

                # s1-s0 row for this head -> tanh((s1-s0)/2 * SCALE).
                kd = ppair.tile([128, 1], BF16, tag="kd", name="kd")
                nc.vector.memset(kd[:], 0.0)
                nc.vector.tensor_tensor(kd[r0:r0 + 64, 0:1], kt[r0:r0 + 64, 1:2],
                                        kt[r0:r0 + 64, 0:1], ALU.subtract)
                th = ppair.tile([1, 1024], BF16, tag="th", name="th")
                for c in range(2):
                    d_ps = pps.tile([1, 512], F32, tag="small", name="d_ps",
                                    bufs=1)
                    nc.tensor.matmul(d_ps[:], kd[:],
                                     qt[:, LQ0 + 512 * c:LQ0 + 512 * c + 512],
                                     start=True, stop=True)
                    nc.scalar.activation(th[:, 512 * c:512 * c + 512], d_ps[:],
                                         AF.Tanh, scale=SCALE * 0.5)

                # global stats for this head: [2 g, 65] (col 64 = sum-exp)
                ps_wv = pps.tile([2, 65], F32, tag="small", name="ps_wv",
                                 bufs=1)
                for t in range(1, 9):
                    pslice = bass.AP(pgp.tensor,
                                     pgp[:].offset + 4 * (t - 1) + 2 * parity,
                                     [[pgp.ap[0][0], 128], [1, 2]])
                    vslice = bass.AP(v96[t].tensor,
                                     v96[t][:].offset + 96 * hl,
                                     [[v96[t].ap[0][0], 128], [1, 65]])
                    nc.tensor.matmul(ps_wv[:], pslice, vslice,
                                     start=(t == 1), stop=(t == 8))
                nc.scalar.copy(gst[0:2, 96 * h:96 * h + 65], ps_wv[:])

                # window scores -> exp (scale + edge-mask bias folded in)
                # -> pt [128 kt, 3840]
                pt = ppt.tile([128, 3840], BF16, tag="pt", name="pt")
                for t in range(NW):
                    qlo, qhi = max(t - 1, 1), min(t + 1, 8)
                    n = (qhi - qlo + 1) * 128
                    ps_s = pps.tile([128, 384], F32, tag="pss", name="ps_s",
                                    bufs=3)
                    nc.tensor.matmul(ps_s[:, :n], kh[:, C(t):C(t) + 128],
                                     qh[:, C(qlo):C(qlo) + n],
                                     start=True, stop=True)
                    col = ptcol(t, qlo)
                    bias = ml_sb[:] if t == 0 else (
                        mr_sb[:] if t == NW - 1 else 0.0)
                    nc.scalar.activation(pt[:, col:col + n], ps_s[:, :n],
                                         AF.Exp, scale=SCALE, bias=bias)
                # P.V with ones-column denominator
                ps_ob = [pps.tile([96, 512], F32, tag="pso", name=f"po{bank}",
                                  bufs=2)
                         for bank in range(2)]
                for bank in range(2):
                    for (t, qs, nb, st, sp) in PV_SCHED[bank]:
                        c0 = 128 * (qs - 1) - 512 * bank
                        nc.tensor.matmul(
                            ps_ob[bank][:, c0:c0 + 128 * nb],
                            v96[t][:, 96 * hl:96 * hl + 96],
                            pt[:, ptcol(t, qs):ptcol(t, qs) + 128 * nb],
                            start=st, stop=sp)
                # xg via tanh identity: out_xg = vs + tanh(d/2)*vd
                # (the vs term rides the epilogue as a per-partition scalar)
                vs_cols, vd_row = vrows
                ps_oxb = []
                for c in range(2):
                    oxb = pps.tile([64, 512], F32, tag="small", name=f"oxb{c}",
                                   bufs=1)
                    nc.tensor.matmul(oxb[:],
                                     vd_row[:, 64 * hl:64 * hl + 64],
                                     th[:, 512 * c:512 * c + 512],
                                     start=True, stop=True)
                    ps_oxb.append(oxb)

                # epilogue: den broadcast to 64 partitions via shuffles,
                # 1/den, then at = ps_ob * bl + (vs + ps_oxb)
                bl = psm.tile([64, 1024], F32, tag="bl", name="bl")
                for bank in range(2):
                    sl = slice(512 * bank, 512 * bank + 512)
                    nc.vector.stream_shuffle(bl[0:32, sl], ps_ob[bank][64:96, :],
                                             [0] * 32)
                    nc.vector.stream_shuffle(bl[32:64, sl], ps_ob[bank][64:96, :],
                                             [0] * 32)
                nc.vector.reciprocal_approx_fast(bl[:], bl[:])
                tmp = psm.tile([64, 1024], BF16, tag="tmp", name="tmp")
                for bank in range(2):
                    sl = slice(512 * bank, 512 * bank + 512)
                    nc.vector.tensor_tensor(tmp[:, sl], ps_ob[bank][0:64, :],
                                            bl[:, sl], ALU.mult)
                vsc = vs_cols[0:64, hl:hl + 1]
                for c in range(2):
                    sl = slice(512 * c, 512 * c + 512)
                    nc.vector.scalar_tensor_tensor(
                        at_sb[h // 2][r0:r0 + 64, sl],
                        tmp[:, sl], vsc, ps_oxb[c][:],
                        ALU.add, ALU.add)

            # software-pipelined emission: pass-B q/k projections interleave
            # with pass-A attention head groups
            qk0, qk1 = {}, {}
            emit_qk_proj(0, 0, qk0)
            emit_qk_proj(0, 1, qk0)
            v96_0, vr0 = emit_v_proj(0)
            for h in range(0, 4):
                if h % 2 == 0:
                    pgp = emit_pair(0, h // 2, qk0)
                emit_head(h, qk0, v96_0, vr0, pgp)
            emit_qk_proj(1, 0, qk1)
            for h in range(4, 8):
                if h % 2 == 0:
                    pgp = emit_pair(0, h // 2, qk0)
                emit_head(h, qk0, v96_0, vr0, pgp)
            emit_qk_proj(1, 1, qk1)
            v96_1, vr1 = emit_v_proj(1)
            for h in range(8, 16):
                if h % 2 == 0:
                    pgp = emit_pair(1, (h - 8) // 2, qk1)
                emit_head(h, qk1, v96_1, vr1, pgp)

            # ================= output projection =================
            # prefetch the first weight tiles before the barrier so their DMAs
            # land during the attention tail
            bo_sb = pc.tile([128, 8], F32, tag="bo")
            nc.sync.dma_start(bo_sb[:], bo)
            wot_pre = []
            for m in range(2):
                wotp = pw.tile([128, 1024], BF16, tag="wo", bufs=4,
                               name=f"wot{m}")
                nc.sync.dma_start(wotp[:], flat2(wo[m]))
                wot_pre.append(wotp)
            tc.no_sync_barrier()
            for m in range(8):
                ps_op = [pps.tile([128, 512], F32, tag=("psj", "pso")[c],
                                  name=f"pop{c}", bufs=2)
                         for c in range(2)]
                if m < 2:
                    wot = wot_pre[m]
                else:
                    wot = pw.tile([128, 1024], BF16, tag="wo", bufs=4)
                    nc.sync.dma_start(wot[:], flat2(wo[m]))
                for f in range(8):
                    for c in range(2):
                        nc.tensor.matmul(ps_op[c][:], wot[:, 128 * f:128 * f + 128],
                                         at_sb[f][:, 512 * c:512 * c + 512],
                                         start=(f == 0), stop=(f == 7))
                ot = pout.tile([128, 1024], F32, tag="ot")
                for c in range(2):
                    nc.scalar.activation(ot[:, 512 * c:512 * c + 512],
                                         ps_op[c][:], AF.Identity,
                                         bias=bo_sb[:, m:m + 1])
                nc.sync.dma_start(outt[128 * m:128 * (m + 1), :], ot[:])
            nc.sync.dma_start(gstats, gst[:])
    return nc


_NC_CACHE = {}
LAST = {}


def get_nc():
    if "nc" not in _NC_CACHE:
        nc = bacc.Bacc("TRN2", target_bir_lowering=False, debug=False, num_devices=8)
        build_kernel(nc)
        nc.compile()
        _NC_CACHE["nc"] = nc
    return _NC_CACHE["nc"]


def make_inputs(x, Wq, Wk, Wv, Wo, bo):
    """Build the 8 per-core input maps (all host-side numpy)."""
    x = np.asarray(x, np.float32)
    Wq = np.asarray(Wq, np.float32)
    Wk = np.asarray(Wk, np.float32)
    Wv = np.asarray(Wv, np.float32)
    Wo = np.asarray(Wo, np.float32)
    bo = np.asarray(bo, np.float32)

    # [blk, partition, sub-blk, col] so each DMA'd SBUF row is contiguous:
    # wq2[ft, p, d, q] = Wq.T[128d+p, 128ft+q]
    wq_r = np.ascontiguousarray(
        Wq.T.reshape(8, 128, 8, 128).transpose(2, 1, 0, 3)).astype(BF)
    wk_r = np.ascontiguousarray(
        Wk.T.reshape(8, 128, 8, 128).transpose(2, 1, 0, 3)).astype(BF)
    wv_r = np.ascontiguousarray(
        Wv.T.reshape(8, 128, 2, 512).transpose(2, 1, 0, 3)).astype(BF)
    wo_r = np.ascontiguousarray(
        Wo.T.reshape(8, 128, 8, 128).transpose(2, 1, 0, 3)).astype(BF)

    bo_r = np.ascontiguousarray(bo.reshape(8, 128).T)
    mask0 = np.zeros((128, 1), np.float32)
    maskneg = np.full((128, 1), -30.0, np.float32)
    in_maps = []
    for core in range(8):
        b, j = divmod(core, 4)
        xs = np.zeros((XCOLS, D_MODEL), np.float32)
        xs[0] = x[b, 0]
        xs[1] = x[b, T - 1]
        xs[TOKS] = 0.5 * x[b, 0]
        xs[TOKS + 1] = 0.5 * x[b, T - 1]
        for w in range(NW):
            gb = 8 * j - 1 + w
            if 0 <= gb < NB:
                xs[2 + 128 * w:2 + 128 * (w + 1)] = x[b, 1 + 128 * gb:1 + 128 * (gb + 1)]
        in_maps.append({
            "xt": np.ascontiguousarray(xs.T).astype(BF),
            "wq": wq_r, "wk": wk_r, "wv": wv_r, "wo": wo_r, "bo": bo_r,
            "maskl": maskneg if j == 0 else mask0,
            "maskr": maskneg if j == 3 else mask0,
        })
    return in_maps


def assemble_output(results, x, Wq, Wk, Wv, Wo, bo):
    x = np.asarray(x, np.float32)
    out = np.empty((B, T, D_MODEL), np.float32)
    for core in range(8):
        b, j = divmod(core, 4)
        out[b, 1 + 1024 * j:1 + 1024 * (j + 1), :] = results[core]["outt"].T

    # global token rows, exact on host
    xg = x[:, [0, T - 1], :]                      # [B, 2, D]
    qg = (xg @ Wq.T).reshape(B, 2, H, DK) * SCALE  # [B, 2, H, DK]
    kg = (xg @ Wk.T).reshape(B, 2, H, DK)
    vg = (xg @ Wv.T).reshape(B, 2, H, DK)
    for b in range(B):
        se = np.zeros((H, 2))
        wvs = np.zeros((H, 2, DK))
        for j in range(4):
            g = results[4 * b + j]["gstats"]  # [2, 1536]
            for h in range(H):
                for gi in range(2):
                    se[h, gi] += g[gi, 96 * h + 64]
                    wvs[h, gi] += g[gi, 96 * h:96 * h + 64]
        # add the global-key terms: scores qg . kg
        sgg = np.einsum("ghd,fhd->hgf", qg[b], kg[b])  # [H, 2g(query), 2f(key)]
        egg = np.exp(sgg)
        num = wvs + np.einsum("hgf,fhd->hgd", egg, vg[b])
        den = se + egg.sum(-1)
        og = num / den[..., None]                  # [H, 2, DK]
        for gi, trow in ((0, 0), (1, T - 1)):
            row = og[:, gi, :].reshape(H * DK)
            out[b, trow] = row @ Wo.T + bo
    return out


def kernel(x, Wq, Wk, Wv, Wo, bo):
    nc = get_nc()
    in_maps = make_inputs(x, Wq, Wk, Wv, Wo, bo)
    res = run_bass_kernel_spmd(nc, in_maps, core_ids=list(range(8)))
    LAST["res"] = res
    results = [{k: np.asarray(v) for k, v in r.items()} for r in res.results]
    return assemble_output(results, x, Wq, Wk, Wv, Wo, bo)
